# revision 14
# baseline (speedup 1.0000x reference)
"""DeepISTA (100 unrolled FISTA iterations) on 8 TRN2 NeuronCores.

Problem: y (256, 16384) f32, D (256, 512) f32 ->
         out = z_100 (512, 16384) f32 from

    out_k = softshrink(z_{k-1} - step*D^T(D z_{k-1} - y), thr)
    z_k   = out_k + mu_k (out_k - out_{k-1})

Sharding: data-parallel over samples (16384 -> 8 x 2048); D replicated.
No inter-core communication is needed (pure forward iteration).

Residual formulation (kept deliberately: HW f32r has only ~11 effective
mantissa bits, and D^T D is rank-256, so any scheme that routes the
identity part of the gradient step through an f32r matmul compounds
rounding on null(D) to ~2e-2; here roundings scale with the decaying
residual r = D z - y and the identity path rides FSHRINK's exact-fp32
in1 = z, giving ~6e-4 end-to-end):

    psum_r = D @ zr - y      (PE f32r MM1 + negI-stationary y-fold)
    r      = copy(psum_r)    (ACT, PSUM->SBUF, f32r)
    psum_u = (step*D)^T @ r  (PE f32r MM2)
    Otil   = FSHRINK(psum_u, z)  (DVE: x = c*(psum-z); x - clip(x, +-c*thr))
    sb     = beta_k * Otil_old   (ACT, in place)
    z      = sb - Otil           (Pool, exact fp32)
    zr     = f32r(z)             (Pool copy; feeds MM1)

with Otil_k = -(1+mu_k) out_k and per-iteration scalars as immediates.
Engine placement is tuned to measured HW rates (PE ~0.65x the CoreSim
cost model, ACT ~1x, DVE ~2.4x SLOWER): DVE runs nothing but the 8
FSHRINKs; all y-subs fold on the PE (+8 cheap matmuls), z-copies and
momentum subs on Pool, r-copies and momentum scales on ACT.
"""

import sys

if "/opt/trn_rl_repo" not in sys.path:
    sys.path.insert(0, "/opt/trn_rl_repo")

import numpy as np

# ---------------------------------------------------------------- constants
N_ITER = 100
LAMBD = 0.1
LIPSCHITZ = 8.0
DIM_Y, DIM_X, N_SAMPLES = 256, 512, 16384
N_CORES = 8
NSH = N_SAMPLES // N_CORES  # 2048 samples per core

# per-(nh*4+mx) chunk engine choices, tuned to balance DVE/ACT/GPSIMD:
MOM_ON_GP = (True, True, True, True, True, True, True, True)  # all ACT+gp
WEIGHT_MAJOR = False  # pair consecutive matmuls on the same stationary operand
# HW: DVE runs ~2.4x slower than the cost model while ACT ~1x and PE ~0.65x,
# so DVE must do nothing but the 8 FSHRINKs: y-subs all fold on the PE
# (negI matmul + ACT psum->sbuf copy), z-copies all on Pool.
YSUB_ON_DVE = (False,) * 8
# Pool handles 16 [128,1024] ops/iter (momentum subs + z-copies) and is a
# co-bottleneck with DVE; push 4 z-copies to ACT which has slack.
ZCOPY_ENG = ("a", "g", "a", "v", "a", "g", "a", "v")

_BUILD_CACHE = {}


# ------------------------------------------------------- custom DVE op
def _get_fshrink():
    import concourse.dve_ops as dve_ops_mod
    from concourse.dve_ops import DveOp
    from concourse.dve_spec import Spec, Src0, Src1, C0, C1, C2, maxx, minn, lower
    from concourse.dve_uop import DveOpSpec

    if any(op.name == "FSHRINK_ANT" for op in dve_ops_mod.OPS):
        return next(op for op in dve_ops_mod.OPS if op.name == "FSHRINK_ANT")

    def _ref(in0, in1, s0, s1, imm2):
        x = (s0 * (in0 - in1)).astype(np.float32)
        return (x - np.clip(x, s1, imm2)).astype(np.float32)

    x = (Src0 - Src1) * C0
    body = x - minn(maxx(x, C1), C2)
    spec = Spec(body=body, reference=_ref)

    row = max(dve_ops_mod._SUB_OPCODE_FOR_NAME.values()) + 1
    assert row < 0x20
    shas = {}
    for ver in ("v3",):
        uops = lower(spec, ver=ver)
        shas[ver] = DveOpSpec(
            name="FSHRINK_ANT", opcode=row, uops=uops, rd1_en=True
        ).sha(ver)
    op = DveOp("FSHRINK_ANT", spec, subdim=False, uops_sha=shas)
    dve_ops_mod.OPS.append(op)
    dve_ops_mod.CUSTOM_DVE_SPECS[op.name] = op.spec
    dve_ops_mod._SUB_OPCODE_FOR_NAME[op.name] = row
    return op


# ------------------------------------------------------- iteration scalars
def _fista_scalars(n_iter):
    """Replicate the reference's fp32 t-recurrence exactly."""
    f32 = np.float32
    t_old = f32(1.0)
    mus = []
    for _ in range(n_iter):
        t = f32(0.5) * (f32(1.0) + np.sqrt(f32(1.0) + f32(4.0) * t_old * t_old))
        mus.append((t_old - f32(1.0)) / t)
        t_old = t
    cs = [f32(1.0) + m for m in mus]  # 1+mu_k
    betas = [f32(0.0)] + [mus[k] / cs[k - 1] for k in range(1, n_iter)]
    step = f32(1.0) / f32(LIPSCHITZ)
    thr = step * f32(LAMBD)
    cthrs = [c * thr for c in cs]
    return cs, betas, cthrs


# ------------------------------------------------------- bass module build
def _build(n_iter=N_ITER):
    key = (n_iter, WEIGHT_MAJOR, YSUB_ON_DVE, MOM_ON_GP, ZCOPY_ENG)
    if key in _BUILD_CACHE:
        return _BUILD_CACHE[key]

    from concourse import bacc
    import concourse.mybir as mybir
    import concourse.tile as tile

    FSHRINK = _get_fshrink()
    F32 = mybir.dt.float32
    F32R = mybir.dt.float32r
    ALU = mybir.AluOpType
    ACTF = mybir.ActivationFunctionType

    cs, betas, cthrs = _fista_scalars(n_iter)

    nc = bacc.Bacc()
    y_d = nc.dram_tensor("y", [DIM_Y, NSH], F32R, kind="ExternalInput")
    dt_d = nc.dram_tensor("dt", [DIM_X, DIM_Y], F32R, kind="ExternalInput")  # D^T
    sd_d = nc.dram_tensor("sd", [DIM_Y, DIM_X], F32R, kind="ExternalInput")  # step*D
    ni_d = nc.dram_tensor("ni", [128, 128], F32R, kind="ExternalInput")  # -I
    out_d = nc.dram_tensor("out", [DIM_X, NSH], F32, kind="ExternalOutput")

    with tile.TileContext(nc) as tc:
        with (
            tc.tile_pool(name="sb", bufs=1) as sb,
            tc.tile_pool(name="pr", bufs=4, space="PSUM") as prp,
            tc.tile_pool(name="pu", bufs=2, space="PSUM") as pup,
        ):
            y2 = [sb.tile([128, NSH], F32R, tag=f"y{m}", name=f"y{m}") for m in range(2)]
            r2 = [sb.tile([128, NSH], F32R, tag=f"r{m}", name=f"r{m}") for m in range(2)]
            # z state: single-buffered fp32 (read by FSHRINK before its
            # in-place overwrite by the momentum op); z_r: f32r matmul copy
            z32 = [sb.tile([128, NSH], F32, tag=f"z{i}", name=f"z{i}") for i in range(4)]
            zr = [sb.tile([128, NSH], F32R, tag=f"zr{i}", name=f"zr{i}") for i in range(4)]
            Oa = [sb.tile([128, NSH], F32, tag=f"Oa{i}", name=f"Oa{i}") for i in range(4)]
            Ob = [sb.tile([128, NSH], F32, tag=f"Ob{i}", name=f"Ob{i}") for i in range(4)]
            # D^T packed [128, 4*256]: chunk kx at cols kx*256:(kx+1)*256
            DTt = sb.tile([128, 4 * DIM_Y], F32R, tag="DTt", name="DTt")
            # step*D packed [128, 2*512]: chunk kc at cols kc*512:(kc+1)*512
            sDt = sb.tile([128, 2 * DIM_X], F32R, tag="sDt", name="sDt")
            negI = sb.tile([128, 128], F32R, tag="negI", name="negI")

            # spread initial loads over several HWDGE queues so they run
            # in parallel (the first matmuls need DTt/sDt/zr, not y)
            for i in range(4):
                nc.sync.dma_start(
                    DTt[:, i * DIM_Y : (i + 1) * DIM_Y],
                    dt_d[i * 128 : (i + 1) * 128, :],
                )
            for i in range(2):
                nc.scalar.dma_start(
                    sDt[:, i * DIM_X : (i + 1) * DIM_X],
                    sd_d[i * 128 : (i + 1) * 128, :],
                )
            nc.scalar.dma_start(negI[:], ni_d[:])
            nc.gpsimd.dma_start(y2[0][:], y_d[0:128, :])
            nc.gpsimd.dma_start(y2[1][:], y_d[128:256, :])

            for i in range(4):
                (nc.gpsimd if i % 2 else nc.vector).memset(Oa[i][:], 0.0)

            for k in range(n_iter):
                c_k = float(cs[k])
                beta_k = float(betas[k])
                cthr_k = float(cthrs[k])
                O_old = Oa if k % 2 == 0 else Ob
                O_new = Ob if k % 2 == 0 else Oa

                for nh in range(2):
                    if WEIGHT_MAJOR:
                        # consecutive matmuls share the stationary operand
                        prs = {}
                        for m in range(2):
                            for ns in range(2):
                                prs[(m, ns)] = prp.tile(
                                    [128, 512], F32, tag="pr", name="pr"
                                )
                        if k > 0:
                            for m in range(2):
                                for kx in range(4):
                                    w = DTt[:, kx * 256 + m * 128 : kx * 256 + (m + 1) * 128]
                                    for ns in range(2):
                                        n = nh * 2 + ns
                                        nc.tensor.matmul(
                                            prs[(m, ns)][:],
                                            w,
                                            zr[kx][:, n * 512 : (n + 1) * 512],
                                            start=(kx == 0),
                                            stop=False,
                                        )
                        for m in range(2):
                            for ns in range(2):
                                n = nh * 2 + ns
                                nc.tensor.matmul(
                                    prs[(m, ns)][:],
                                    negI[:],
                                    y2[m][:, n * 512 : (n + 1) * 512],
                                    start=(k == 0),
                                    stop=True,
                                )
                                nc.scalar.activation(
                                    r2[m][:, n * 512 : (n + 1) * 512],
                                    prs[(m, ns)][:],
                                    ACTF.Copy,
                                )
                    else:
                        for ns in range(2):
                            n = nh * 2 + ns
                            nsl = slice(n * 512, (n + 1) * 512)
                            for m in range(2):
                                dve_ysub = YSUB_ON_DVE[ns * 2 + m + nh * 4] and k > 0
                                pr = prp.tile([128, 512], F32, tag="pr", name="pr")
                                if k > 0:
                                    for kx in range(4):
                                        nc.tensor.matmul(
                                            pr[:],
                                            DTt[:, kx * 256 + m * 128 : kx * 256 + (m + 1) * 128],
                                            zr[kx][:, nsl],
                                            start=(kx == 0),
                                            stop=(kx == 3) if dve_ysub else False,
                                        )
                                if dve_ysub:
                                    # r2 = psum - y on DVE, skipping the PE fold
                                    nc.vector.scalar_tensor_tensor(
                                        r2[m][:, nsl],
                                        y2[m][:, nsl].bitcast(F32),
                                        -1.0,
                                        pr[:],
                                        op0=ALU.mult,
                                        op1=ALU.add,
                                    )
                                else:
                                    nc.tensor.matmul(
                                        pr[:],
                                        negI[:],
                                        y2[m][:, nsl],
                                        start=(k == 0),
                                        stop=True,
                                    )
                                    nc.scalar.activation(r2[m][:, nsl], pr[:], ACTF.Copy)
                    hsl = slice(nh * 1024, (nh + 1) * 1024)
                    for mx in range(4):
                        ci = nh * 4 + mx
                        pu = pup.tile([128, 1024], F32, tag="pu", name="pu")
                        if WEIGHT_MAJOR:
                            for kc in range(2):
                                w = sDt[:, kc * 512 + mx * 128 : kc * 512 + (mx + 1) * 128]
                                for ns in range(2):
                                    n = nh * 2 + ns
                                    nc.tensor.matmul(
                                        pu[:, ns * 512 : (ns + 1) * 512],
                                        w,
                                        r2[kc][:, n * 512 : (n + 1) * 512],
                                        start=(kc == 0),
                                        stop=(kc == 1),
                                    )
                        else:
                            for ns in range(2):
                                n = nh * 2 + ns
                                nsl = slice(n * 512, (n + 1) * 512)
                                psl = slice(ns * 512, (ns + 1) * 512)
                                for kc in range(2):
                                    nc.tensor.matmul(
                                        pu[:, psl],
                                        sDt[:, kc * 512 + mx * 128 : kc * 512 + (mx + 1) * 128],
                                        r2[kc][:, nsl],
                                        start=(kc == 0),
                                        stop=(kc == 1),
                                    )
                        nc.vector._custom_dve(
                            FSHRINK,
                            out=O_new[mx][:, hsl],
                            in0=pu[:],
                            in1=(
                                z32[mx][:, hsl]
                                if k > 0
                                else O_old[mx][:, hsl]  # zeros at k == 0
                            ),
                            s0=c_k,
                            s1=-cthr_k,
                            imm2=cthr_k,
                        )
                        # momentum: z32 <- beta*O_old - O_new (in place)
                        if MOM_ON_GP[ci]:
                            # no stt on gpsimd; ACT scales O_old in place,
                            # gpsimd does the subtract
                            nc.scalar.activation(
                                O_old[mx][:, hsl],
                                O_old[mx][:, hsl],
                                ACTF.Copy,
                                bias=0.0,
                                scale=beta_k,
                            )
                            nc.gpsimd.tensor_tensor(
                                z32[mx][:, hsl],
                                O_old[mx][:, hsl],
                                O_new[mx][:, hsl],
                                op=ALU.subtract,
                            )
                        else:
                            nc.vector.scalar_tensor_tensor(
                                z32[mx][:, hsl],
                                O_old[mx][:, hsl],
                                beta_k,
                                O_new[mx][:, hsl],
                                op0=ALU.mult,
                                op1=ALU.subtract,
                            )
                        # f32r copy for next iteration's MM1 (skip on last)
                        if k < n_iter - 1:
                            e = ZCOPY_ENG[ci]
                            if e == "v":
                                nc.vector.tensor_copy(
                                    zr[mx][:, hsl], z32[mx][:, hsl]
                                )
                            elif e == "a":
                                nc.scalar.activation(
                                    zr[mx][:, hsl], z32[mx][:, hsl], ACTF.Copy
                                )
                            else:
                                nc.gpsimd.tensor_copy(
                                    zr[mx][:, hsl], z32[mx][:, hsl]
                                )

            out_q = (nc.sync, nc.scalar, nc.gpsimd, nc.sync)
            for i in range(4):
                out_q[i].dma_start(out_d[i * 128 : (i + 1) * 128, :], z32[i][:])

    nc.compile()
    _BUILD_CACHE[key] = nc
    return nc


# ------------------------------------------------------- host-side driver
def _host_inputs(y, D):
    DT = np.ascontiguousarray(D.T.astype(np.float32))
    sD = np.ascontiguousarray((np.float32(1.0 / LIPSCHITZ) * D).astype(np.float32))
    negI = (-np.eye(128)).astype(np.float32)
    in_maps = []
    for c in range(N_CORES):
        ysh = np.ascontiguousarray(y[:, c * NSH : (c + 1) * NSH].astype(np.float32))
        in_maps.append({"y": ysh, "dt": DT, "sd": sD, "ni": negI})
    return in_maps


LAST_EXEC_NS = None


def kernel(y, D):
    global LAST_EXEC_NS
    import os

    from concourse.bass_utils import run_bass_kernel_spmd

    y = np.asarray(y, dtype=np.float32)
    D = np.asarray(D, dtype=np.float32)
    assert y.shape == (DIM_Y, N_SAMPLES) and D.shape == (DIM_Y, DIM_X)

    nc = _build(N_ITER)
    in_maps = _host_inputs(y, D)
    trace = os.environ.get("DEEPISTA_TRACE", "0") == "1"
    r = run_bass_kernel_spmd(nc, in_maps, list(range(N_CORES)), trace=trace)
    LAST_EXEC_NS = r.exec_time_ns
    out = np.concatenate([r.results[c]["out"] for c in range(N_CORES)], axis=1)
    return out.astype(np.float32)



# revision 16
# speedup vs baseline: 1.3659x; 1.3659x over previous
"""DeepISTA (100 unrolled FISTA iterations) on 8 TRN2 NeuronCores.

Problem: y (256, 16384) f32, D (256, 512) f32 ->
         out = z_100 (512, 16384) f32 from

    out_k = softshrink(z_{k-1} - step*D^T(D z_{k-1} - y), thr)
    z_k   = out_k + mu_k (out_k - out_{k-1})

Sharding: data-parallel over samples (16384 -> 8 x 2048); D replicated.
No inter-core communication is needed (pure forward iteration).

Residual formulation (kept deliberately: HW f32r has only ~11 effective
mantissa bits, and D^T D is rank-256, so any scheme that routes the
identity part of the gradient step through an f32r matmul compounds
rounding on null(D) to ~2e-2; here roundings scale with the decaying
residual r = D z - y and the identity path rides FSHRINK's exact-fp32
in1 = z, giving ~6e-4 end-to-end):

    psum_r = D @ zr - y      (PE f32r MM1 + negI-stationary y-fold)
    r      = copy(psum_r)    (ACT, PSUM->SBUF, f32r)
    psum_u = (step*D)^T @ r  (PE f32r MM2)
    Otil   = FSHRINK(psum_u, z)  (DVE: x = c*(psum-z); x - clip(x, +-c*thr))
    sb     = beta_k * Otil_old   (ACT, in place)
    z      = sb - Otil           (Pool, exact fp32)
    zr     = f32r(z)             (Pool copy; feeds MM1)

with Otil_k = -(1+mu_k) out_k and per-iteration scalars as immediates.
Engine placement is tuned to measured HW rates (PE ~0.65x the CoreSim
cost model, ACT ~1x, DVE ~2.4x SLOWER): DVE runs nothing but the 8
FSHRINKs; all y-subs fold on the PE (+8 cheap matmuls), z-copies and
momentum subs on Pool, r-copies and momentum scales on ACT.
"""

import sys

if "/opt/trn_rl_repo" not in sys.path:
    sys.path.insert(0, "/opt/trn_rl_repo")

import numpy as np

# ---------------------------------------------------------------- constants
N_ITER = 100
LAMBD = 0.1
LIPSCHITZ = 8.0
DIM_Y, DIM_X, N_SAMPLES = 256, 512, 16384
N_CORES = 8
NSH = N_SAMPLES // N_CORES  # 2048 samples per core

# per-(nh*4+mx) chunk engine choices, tuned to balance DVE/ACT/GPSIMD:
MOM_ON_GP = (True, True, False, True, True, True, False, True)  # 2 DVE stt
WEIGHT_MAJOR = False  # pair consecutive matmuls on the same stationary operand
# HW: DVE runs ~2.4x slower than the cost model while ACT ~1x and PE ~0.65x,
# so DVE must do nothing but the 8 FSHRINKs: y-subs all fold on the PE
# (negI matmul + ACT psum->sbuf copy), z-copies all on Pool.
YSUB_ON_DVE = (False,) * 8
# Pool handles 16 [128,1024] ops/iter (momentum subs + z-copies) and is a
# co-bottleneck with DVE; push 4 z-copies to ACT which has slack.
ZCOPY_ENG = ("a", "g", "a", "v", "a", "g", "a", "v")

_BUILD_CACHE = {}


# ------------------------------------------------------- custom DVE op
def _get_fshrink():
    import concourse.dve_ops as dve_ops_mod
    from concourse.dve_ops import DveOp
    from concourse.dve_spec import Spec, Src0, Src1, C0, C1, C2, maxx, minn, lower
    from concourse.dve_uop import DveOpSpec

    if any(op.name == "FSHRINK_ANT" for op in dve_ops_mod.OPS):
        return next(op for op in dve_ops_mod.OPS if op.name == "FSHRINK_ANT")

    def _ref(in0, in1, s0, s1, imm2):
        x = (s0 * (in0 - in1)).astype(np.float32)
        return (x - np.clip(x, s1, imm2)).astype(np.float32)

    x = (Src0 - Src1) * C0
    body = x - minn(maxx(x, C1), C2)
    spec = Spec(body=body, reference=_ref)

    row = max(dve_ops_mod._SUB_OPCODE_FOR_NAME.values()) + 1
    assert row < 0x20
    shas = {}
    for ver in ("v3",):
        uops = lower(spec, ver=ver)
        shas[ver] = DveOpSpec(
            name="FSHRINK_ANT", opcode=row, uops=uops, rd1_en=True
        ).sha(ver)
    op = DveOp("FSHRINK_ANT", spec, subdim=False, uops_sha=shas)
    dve_ops_mod.OPS.append(op)
    dve_ops_mod.CUSTOM_DVE_SPECS[op.name] = op.spec
    dve_ops_mod._SUB_OPCODE_FOR_NAME[op.name] = row
    return op


# ------------------------------------------------------- iteration scalars
def _fista_scalars(n_iter):
    """Replicate the reference's fp32 t-recurrence exactly."""
    f32 = np.float32
    t_old = f32(1.0)
    mus = []
    for _ in range(n_iter):
        t = f32(0.5) * (f32(1.0) + np.sqrt(f32(1.0) + f32(4.0) * t_old * t_old))
        mus.append((t_old - f32(1.0)) / t)
        t_old = t
    cs = [f32(1.0) + m for m in mus]  # 1+mu_k
    betas = [f32(0.0)] + [mus[k] / cs[k - 1] for k in range(1, n_iter)]
    step = f32(1.0) / f32(LIPSCHITZ)
    thr = step * f32(LAMBD)
    cthrs = [c * thr for c in cs]
    return cs, betas, cthrs


# ------------------------------------------------------- bass module build
def _build(n_iter=N_ITER):
    key = (n_iter, WEIGHT_MAJOR, YSUB_ON_DVE, MOM_ON_GP, ZCOPY_ENG)
    if key in _BUILD_CACHE:
        return _BUILD_CACHE[key]

    from concourse import bacc
    import concourse.mybir as mybir
    import concourse.tile as tile

    FSHRINK = _get_fshrink()
    F32 = mybir.dt.float32
    F32R = mybir.dt.float32r
    ALU = mybir.AluOpType
    ACTF = mybir.ActivationFunctionType

    cs, betas, cthrs = _fista_scalars(n_iter)

    nc = bacc.Bacc()
    y_d = nc.dram_tensor("y", [DIM_Y, NSH], F32R, kind="ExternalInput")
    dt_d = nc.dram_tensor("dt", [DIM_X, DIM_Y], F32R, kind="ExternalInput")  # D^T
    sd_d = nc.dram_tensor("sd", [DIM_Y, DIM_X], F32R, kind="ExternalInput")  # step*D
    ni_d = nc.dram_tensor("ni", [128, 128], F32R, kind="ExternalInput")  # -I
    out_d = nc.dram_tensor("out", [DIM_X, NSH], F32, kind="ExternalOutput")

    with tile.TileContext(nc) as tc:
        with (
            tc.tile_pool(name="sb", bufs=1) as sb,
            tc.tile_pool(name="pr", bufs=4, space="PSUM") as prp,
            tc.tile_pool(name="pu", bufs=2, space="PSUM") as pup,
        ):
            y2 = [sb.tile([128, NSH], F32R, tag=f"y{m}", name=f"y{m}") for m in range(2)]
            r2 = [sb.tile([128, NSH], F32R, tag=f"r{m}", name=f"r{m}") for m in range(2)]
            # z state: single-buffered fp32 (read by FSHRINK before its
            # in-place overwrite by the momentum op); z_r: f32r matmul copy
            z32 = [sb.tile([128, NSH], F32, tag=f"z{i}", name=f"z{i}") for i in range(4)]
            zr = [sb.tile([128, NSH], F32R, tag=f"zr{i}", name=f"zr{i}") for i in range(4)]
            Oa = [sb.tile([128, NSH], F32, tag=f"Oa{i}", name=f"Oa{i}") for i in range(4)]
            Ob = [sb.tile([128, NSH], F32, tag=f"Ob{i}", name=f"Ob{i}") for i in range(4)]
            # D^T packed [128, 4*256]: chunk kx at cols kx*256:(kx+1)*256
            DTt = sb.tile([128, 4 * DIM_Y], F32R, tag="DTt", name="DTt")
            # step*D packed [128, 2*512]: chunk kc at cols kc*512:(kc+1)*512
            sDt = sb.tile([128, 2 * DIM_X], F32R, tag="sDt", name="sDt")
            negI = sb.tile([128, 128], F32R, tag="negI", name="negI")

            # spread initial loads over several HWDGE queues so they run
            # in parallel (the first matmuls need DTt/sDt/zr, not y)
            for i in range(4):
                nc.sync.dma_start(
                    DTt[:, i * DIM_Y : (i + 1) * DIM_Y],
                    dt_d[i * 128 : (i + 1) * 128, :],
                )
            for i in range(2):
                nc.scalar.dma_start(
                    sDt[:, i * DIM_X : (i + 1) * DIM_X],
                    sd_d[i * 128 : (i + 1) * 128, :],
                )
            nc.scalar.dma_start(negI[:], ni_d[:])
            nc.gpsimd.dma_start(y2[0][:], y_d[0:128, :])
            nc.gpsimd.dma_start(y2[1][:], y_d[128:256, :])

            for i in range(4):
                (nc.gpsimd if i % 2 else nc.vector).memset(Oa[i][:], 0.0)

            for k in range(n_iter):
                c_k = float(cs[k])
                beta_k = float(betas[k])
                cthr_k = float(cthrs[k])
                O_old = Oa if k % 2 == 0 else Ob
                O_new = Ob if k % 2 == 0 else Oa

                for nh in range(2):
                    if WEIGHT_MAJOR:
                        # consecutive matmuls share the stationary operand
                        prs = {}
                        for m in range(2):
                            for ns in range(2):
                                prs[(m, ns)] = prp.tile(
                                    [128, 512], F32, tag="pr", name="pr"
                                )
                        if k > 0:
                            for m in range(2):
                                for kx in range(4):
                                    w = DTt[:, kx * 256 + m * 128 : kx * 256 + (m + 1) * 128]
                                    for ns in range(2):
                                        n = nh * 2 + ns
                                        nc.tensor.matmul(
                                            prs[(m, ns)][:],
                                            w,
                                            zr[kx][:, n * 512 : (n + 1) * 512],
                                            start=(kx == 0),
                                            stop=False,
                                        )
                        for m in range(2):
                            for ns in range(2):
                                n = nh * 2 + ns
                                nc.tensor.matmul(
                                    prs[(m, ns)][:],
                                    negI[:],
                                    y2[m][:, n * 512 : (n + 1) * 512],
                                    start=(k == 0),
                                    stop=True,
                                )
                                nc.scalar.activation(
                                    r2[m][:, n * 512 : (n + 1) * 512],
                                    prs[(m, ns)][:],
                                    ACTF.Copy,
                                )
                    else:
                        for ns in range(2):
                            n = nh * 2 + ns
                            nsl = slice(n * 512, (n + 1) * 512)
                            for m in range(2):
                                dve_ysub = YSUB_ON_DVE[ns * 2 + m + nh * 4] and k > 0
                                pr = prp.tile([128, 512], F32, tag="pr", name="pr")
                                if k > 0:
                                    for kx in range(4):
                                        nc.tensor.matmul(
                                            pr[:],
                                            DTt[:, kx * 256 + m * 128 : kx * 256 + (m + 1) * 128],
                                            zr[kx][:, nsl],
                                            start=(kx == 0),
                                            stop=(kx == 3) if dve_ysub else False,
                                        )
                                if dve_ysub:
                                    # r2 = psum - y on DVE, skipping the PE fold
                                    nc.vector.scalar_tensor_tensor(
                                        r2[m][:, nsl],
                                        y2[m][:, nsl].bitcast(F32),
                                        -1.0,
                                        pr[:],
                                        op0=ALU.mult,
                                        op1=ALU.add,
                                    )
                                else:
                                    nc.tensor.matmul(
                                        pr[:],
                                        negI[:],
                                        y2[m][:, nsl],
                                        start=(k == 0),
                                        stop=True,
                                    )
                                    nc.scalar.activation(r2[m][:, nsl], pr[:], ACTF.Copy)
                    hsl = slice(nh * 1024, (nh + 1) * 1024)
                    for mx in range(4):
                        ci = nh * 4 + mx
                        pu = pup.tile([128, 1024], F32, tag="pu", name="pu")
                        if WEIGHT_MAJOR:
                            for kc in range(2):
                                w = sDt[:, kc * 512 + mx * 128 : kc * 512 + (mx + 1) * 128]
                                for ns in range(2):
                                    n = nh * 2 + ns
                                    nc.tensor.matmul(
                                        pu[:, ns * 512 : (ns + 1) * 512],
                                        w,
                                        r2[kc][:, n * 512 : (n + 1) * 512],
                                        start=(kc == 0),
                                        stop=(kc == 1),
                                    )
                        else:
                            for ns in range(2):
                                n = nh * 2 + ns
                                nsl = slice(n * 512, (n + 1) * 512)
                                psl = slice(ns * 512, (ns + 1) * 512)
                                for kc in range(2):
                                    nc.tensor.matmul(
                                        pu[:, psl],
                                        sDt[:, kc * 512 + mx * 128 : kc * 512 + (mx + 1) * 128],
                                        r2[kc][:, nsl],
                                        start=(kc == 0),
                                        stop=(kc == 1),
                                    )
                        nc.vector._custom_dve(
                            FSHRINK,
                            out=O_new[mx][:, hsl],
                            in0=pu[:],
                            in1=(
                                z32[mx][:, hsl]
                                if k > 0
                                else O_old[mx][:, hsl]  # zeros at k == 0
                            ),
                            s0=c_k,
                            s1=-cthr_k,
                            imm2=cthr_k,
                        )
                        # momentum: z32 <- beta*O_old - O_new (in place)
                        if MOM_ON_GP[ci]:
                            # no stt on gpsimd; ACT scales O_old in place,
                            # gpsimd does the subtract
                            nc.scalar.activation(
                                O_old[mx][:, hsl],
                                O_old[mx][:, hsl],
                                ACTF.Copy,
                                bias=0.0,
                                scale=beta_k,
                            )
                            nc.gpsimd.tensor_tensor(
                                z32[mx][:, hsl],
                                O_old[mx][:, hsl],
                                O_new[mx][:, hsl],
                                op=ALU.subtract,
                            )
                        else:
                            nc.vector.scalar_tensor_tensor(
                                z32[mx][:, hsl],
                                O_old[mx][:, hsl],
                                beta_k,
                                O_new[mx][:, hsl],
                                op0=ALU.mult,
                                op1=ALU.subtract,
                            )
                        # f32r copy for next iteration's MM1 (skip on last)
                        if k < n_iter - 1:
                            e = ZCOPY_ENG[ci]
                            if e == "v":
                                nc.vector.tensor_copy(
                                    zr[mx][:, hsl], z32[mx][:, hsl]
                                )
                            elif e == "a":
                                nc.scalar.activation(
                                    zr[mx][:, hsl], z32[mx][:, hsl], ACTF.Copy
                                )
                            else:
                                nc.gpsimd.tensor_copy(
                                    zr[mx][:, hsl], z32[mx][:, hsl]
                                )

            out_q = (nc.sync, nc.scalar, nc.gpsimd, nc.sync)
            for i in range(4):
                out_q[i].dma_start(out_d[i * 128 : (i + 1) * 128, :], z32[i][:])

    nc.compile()
    _BUILD_CACHE[key] = nc
    return nc


# ------------------------------------------------------- host-side driver
def _host_inputs(y, D):
    DT = np.ascontiguousarray(D.T.astype(np.float32))
    sD = np.ascontiguousarray((np.float32(1.0 / LIPSCHITZ) * D).astype(np.float32))
    negI = (-np.eye(128)).astype(np.float32)
    in_maps = []
    for c in range(N_CORES):
        ysh = np.ascontiguousarray(y[:, c * NSH : (c + 1) * NSH].astype(np.float32))
        in_maps.append({"y": ysh, "dt": DT, "sd": sD, "ni": negI})
    return in_maps


LAST_EXEC_NS = None


def kernel(y, D):
    global LAST_EXEC_NS
    import os

    from concourse.bass_utils import run_bass_kernel_spmd

    y = np.asarray(y, dtype=np.float32)
    D = np.asarray(D, dtype=np.float32)
    assert y.shape == (DIM_Y, N_SAMPLES) and D.shape == (DIM_Y, DIM_X)

    nc = _build(N_ITER)
    in_maps = _host_inputs(y, D)
    trace = os.environ.get("DEEPISTA_TRACE", "0") == "1"
    r = run_bass_kernel_spmd(nc, in_maps, list(range(N_CORES)), trace=trace)
    LAST_EXEC_NS = r.exec_time_ns
    out = np.concatenate([r.results[c]["out"] for c in range(N_CORES)], axis=1)
    return out.astype(np.float32)

